# revision 3
# baseline (speedup 1.0000x reference)
"""nn_APCNNCrop on 8 Trainium2 NeuronCores (pure data parallelism, 4 samples/core).

Pipeline:
  Phase A (device): channel means of x3/x4/x5 (exact fp32 PE reduction; DVE
        pre-adds the two 128-channel halves, then one basis matmul per sample
        routes each sample's sum to its own PSUM partition).
  Host: greedy NMS per level -> roi3/roi4/roi5, union crop window, and the
        per-sample bilinear interpolation matrices Ry/Rx (crop-resize is
        linear in x2: crop[c] = Ry @ x2[c] @ Rx^T).
  Phase B (device): crop via PE matmuls, channel-half packed at partition
        offsets 0/64 so DMAs span 120 partitions:
          MM1' (f32r): T1T[(c2,x), i] = X2ch^T @ Ry^T  (X block stationary --
                the transpose comes for free from the stationary side)
          MM2 (bf16): out[i, (c2,j)] = T1T^T @ RxT2 (block-diag Rx^T)
        Host pre-swizzles x2 to [s, group, half, y, c, x] and un-swizzles the
        crop so every DMA is contiguous per partition.

kernel(**inputs) takes FULL inputs, shards batch across cores 0-7 internally,
returns (roi3, roi4, roi5, crop) exactly like the reference forward().
"""
import numpy as np
import ml_dtypes
import jax
from jax.sharding import Mesh, PartitionSpec
from jax.experimental.shard_map import shard_map

import concourse.mybir as mybir
from concourse import bacc
from concourse.tile import TileContext
from concourse.bass2jax import _bass_exec_p, install_neuronx_cc_hook, partition_id_tensor

N_CORES = 8
S_PER_CORE = 4
N = 32
C = 256
HW = 56
CG = 64
NG = C // CG
SIZES = {"x3": 3136, "x4": 784, "x5": 196}
IMG = 447.0
NEG = np.float32(-1e30)


class SpmdRunner:
    def __init__(self, nc, n_cores=N_CORES):
        install_neuronx_cc_hook()
        self.nc = nc
        self.n_cores = n_cores
        partition_name = nc.partition_id_tensor.name if nc.partition_id_tensor else None
        in_names, out_names, out_avals, zero_outs = [], [], [], []
        for alloc in nc.m.functions[0].allocations:
            if not isinstance(alloc, mybir.MemoryLocationSet):
                continue
            name = alloc.memorylocations[0].name
            if alloc.kind == "ExternalInput":
                if name != partition_name:
                    in_names.append(name)
            elif alloc.kind == "ExternalOutput":
                out_names.append(name)
                shape = tuple(alloc.tensor_shape)
                dtype = mybir.dt.np(alloc.dtype)
                out_avals.append(jax.core.ShapedArray(shape, dtype))
                zero_outs.append(np.zeros(shape, dtype))
        self.in_names = list(in_names)
        self.out_names = list(out_names)
        n_params = len(in_names)
        all_in_names = list(in_names) + list(out_names)
        if partition_name is not None:
            all_in_names.append(partition_name)

        def _body(*args):
            operands = list(args)
            if partition_name is not None:
                operands.append(partition_id_tensor())
            outs = _bass_exec_p.bind(
                *operands,
                out_avals=tuple(out_avals),
                in_names=tuple(all_in_names),
                out_names=tuple(out_names),
                lowering_input_output_aliases=(),
                sim_require_finite=False,
                sim_require_nnan=False,
                nc=nc,
            )
            return tuple(outs)

        devices = jax.devices()[:n_cores]
        self.mesh = Mesh(np.asarray(devices), ("core",))
        in_specs = (PartitionSpec("core"),) * (n_params + len(out_names))
        out_specs = (PartitionSpec("core"),) * len(out_names)
        self.sharded = jax.jit(
            shard_map(_body, mesh=self.mesh, in_specs=in_specs,
                      out_specs=out_specs, check_rep=False),
            keep_unused=True,
        )
        self._zero_concat = [np.concatenate([z] * n_cores, axis=0) for z in zero_outs]

    def put_inputs(self, in_maps):
        sh = jax.sharding.NamedSharding(self.mesh, PartitionSpec("core"))
        args = []
        for name in self.in_names:
            concat = np.concatenate([in_maps[c][name] for c in range(self.n_cores)], axis=0)
            args.append(jax.device_put(concat, sh))
        for z in self._zero_concat:
            args.append(jax.device_put(z, sh))
        return args

    def run_np(self, in_maps):
        args = self.put_inputs(in_maps)
        outs = [np.asarray(o) for o in self.sharded(*args)]
        return {name: outs[i] for i, name in enumerate(self.out_names)}

    def time(self, in_maps, iters=8, warmup=2):
        import time as _t
        args = self.put_inputs(in_maps)
        for _ in range(warmup):
            jax.block_until_ready(self.sharded(*args))
        ts = []
        for _ in range(iters):
            t0 = _t.perf_counter()
            jax.block_until_ready(self.sharded(*args))
            ts.append(_t.perf_counter() - t0)
        return min(ts)


# ---------------------------------------------------------------- phase A
def build_phase_a(repeat=1):
    nc = bacc.Bacc("TRN2", target_bir_lowering=False, debug=False, num_devices=N_CORES)
    ins, outs = {}, {}
    for name, S in SIZES.items():
        ins[name] = nc.declare_dram_parameter(name, [S_PER_CORE, C, S], mybir.dt.float32, isOutput=False)
        outs[name] = nc.declare_dram_parameter("m" + name[1], [S_PER_CORE, S], mybir.dt.float32, isOutput=True)
    with TileContext(nc) as tc:
        with (
            tc.tile_pool(name="const", bufs=1) as cpool,
            tc.tile_pool(name="xin", bufs=2) as xpool,
            tc.tile_pool(name="acc", bufs=2) as apool,
            tc.tile_pool(name="psum", bufs=4, space="PSUM") as ppool,
        ):
            basis = []
            for s in range(S_PER_CORE):
                b = cpool.tile([128, S_PER_CORE], mybir.dt.float32, tag=f"basis{s}")
                nc.vector.memset(b[:], 0.0)
                nc.vector.memset(b[:, s:s + 1], 1.0)
                basis.append(b)

            def abody(_iv=None):
                for name, S in SIZES.items():
                    x, m = ins[name], outs[name]
                    acc = apool.tile([S_PER_CORE, S], mybir.dt.float32, tag=f"acc_{name}")
                    nc.vector.memset(acc[:], 0.0)
                    for s in range(S_PER_CORE):
                        t = xpool.tile([128, 2, S], mybir.dt.float32, tag=f"t_{name}")
                        (nc.sync if s % 2 else nc.scalar).dma_start(
                            out=t[:], in_=x[s].rearrange("(h c) s -> c h s", h=2))
                        th = xpool.tile([128, S], mybir.dt.float32, tag=f"th_{name}")
                        nc.vector.tensor_add(th[:], t[:, 0, :], t[:, 1, :])
                        for j0 in range(0, S, 512):
                            w = min(512, S - j0)
                            ps = ppool.tile([S_PER_CORE, 512], mybir.dt.float32)
                            nc.tensor.matmul(ps[:, :w], basis[s][:, :], th[:, j0:j0 + w],
                                             start=True, stop=True)
                            nc.vector.tensor_add(acc[:, j0:j0 + w], acc[:, j0:j0 + w], ps[:, :w])
                    nc.scalar.mul(out=acc[:], in_=acc[:], mul=1.0 / C)
                    nc.sync.dma_start(out=m[:, :], in_=acc[:])

            if repeat == 1:
                abody()
            else:
                with tc.For_i(0, repeat, 1) as iv:
                    abody(iv)
    nc.compile()
    return nc


# ---------------------------------------------------------------- phase B (fused_h2)
def build_phase_b(loop_n=1):
    QW = 4
    nc = bacc.Bacc("TRN2", target_bir_lowering=False, debug=False, num_devices=N_CORES)
    x2 = nc.declare_dram_parameter("x2g", [S_PER_CORE, NG, 2, HW, CG // 2, HW], mybir.dt.float32, isOutput=False)
    ryt = nc.declare_dram_parameter("ryt", [S_PER_CORE, HW, HW], mybir.dt.float32, isOutput=False)
    rxt2 = nc.declare_dram_parameter("rxt2", [S_PER_CORE, 112, 112], mybir.dt.bfloat16, isOutput=False)
    crop = nc.declare_dram_parameter("cropg", [S_PER_CORE, NG, 2, HW, CG // 2, HW], mybir.dt.float32, isOutput=True)
    with TileContext(nc) as tc:
        with (
            tc.tile_pool(name="const", bufs=1) as cpool,
            tc.tile_pool(name="xin", bufs=3) as xpool,
            tc.tile_pool(name="t1t", bufs=8) as ttpool,
            tc.tile_pool(name="osb", bufs=2) as opool,
            tc.tile_pool(name="pst", bufs=4, space="PSUM") as pstpool,
            tc.tile_pool(name="ps2", bufs=4, space="PSUM") as ps2pool,
        ):
            ryt128s, rxt2s = [], []
            for s in range(S_PER_CORE):
                r2 = cpool.tile([128, HW], mybir.dt.float32r, tag=f"ryt128_{s}")
                nc.sync.dma_start(out=r2[0:56], in_=ryt[s].bitcast(mybir.dt.float32r))
                nc.sync.dma_start(out=r2[64:120], in_=ryt[s].bitcast(mybir.dt.float32r))
                ryt128s.append(r2)
                rx = cpool.tile([112, 112], mybir.dt.bfloat16, tag=f"rxt2{s}")
                nc.sync.dma_start(out=rx[:], in_=rxt2[s])
                rxt2s.append(rx)

            def body(_iv=None):
                for s in range(S_PER_CORE):
                    for g in range(NG):
                        qa = nc.sync if g % 2 == 0 else nc.scalar
                        qb = nc.scalar if g % 2 == 0 else nc.sync
                        Xd = xpool.tile([128, CG // 2, HW], mybir.dt.float32r, tag="Xd")
                        qa.dma_start(out=Xd[0:56], in_=x2[s, g, 0].bitcast(mybir.dt.float32r))
                        qb.dma_start(out=Xd[64:120], in_=x2[s, g, 1].bitcast(mybir.dt.float32r))
                        osb = opool.tile([128, CG // 2, HW], mybir.dt.float32, tag="osb")
                        for hb in range(2):
                            off = 64 * hb
                            for q in range(CG // 2 // 2 // QW):  # 4 units of QW pairs per half
                                pst = pstpool.tile([112, 56 * QW], mybir.dt.float32, tag="pstf")
                                ps2 = ps2pool.tile([128, 112 * QW], mybir.dt.float32, tag="ps2f")
                                for h in range(QW):
                                    p = QW * q + h
                                    xblk = Xd[off:off + 56, 2 * p:2 * p + 2, :].rearrange("y c x -> y (c x)")
                                    nc.tensor.matmul(pst[:, 56 * h:56 * h + 56], xblk,
                                                     ryt128s[s][off:off + 56, :],
                                                     start=True, stop=True)
                                t1t = ttpool.tile([112, 56 * QW], mybir.dt.bfloat16, tag="t1t")
                                if q % 2 == 0:
                                    nc.scalar.copy(out=t1t[:], in_=pst[:])
                                else:
                                    nc.vector.tensor_copy(t1t[:], pst[:])
                                for h in range(QW):
                                    nc.tensor.matmul(ps2[off:off + 56, 112 * h:112 * h + 112],
                                                     t1t[:, 56 * h:56 * h + 56], rxt2s[s][:],
                                                     start=True, stop=True)
                                oblk = osb[off:off + 56, 2 * QW * q:2 * QW * (q + 1), :].rearrange("i c j -> i (c j)")
                                if q % 2 == 0:
                                    nc.vector.tensor_copy(oblk, ps2[off:off + 56, :])
                                else:
                                    nc.scalar.copy(out=oblk, in_=ps2[off:off + 56, :])
                        for hb in range(2):
                            off = 64 * hb
                            q = qa if hb == 0 else qb
                            q.dma_start(out=crop[s, g, hb, :, 0:16, :], in_=osb[off:off + 56, 0:16, :])
                            q.dma_start(out=crop[s, g, hb, :, 16:32, :], in_=osb[off:off + 56, 16:32, :])

            if loop_n == 1:
                body()
            else:
                with tc.For_i(0, loop_n, 1) as iv:
                    body(iv)
    nc.compile()
    return nc


# ---------------------------------------------------------------- host NMS
def _anchors(h, stride, size):
    k = np.arange(h * h, dtype=np.int64)
    cx = (k % h).astype(np.float32) * np.float32(stride)
    cy = (k // h).astype(np.float32) * np.float32(stride)
    half = np.float32(0.5 * size)
    return np.stack([cx - half, cy - half, cx + half, cy + half], axis=1)


def _nms_level(scores, h, stride, size, iou_thr, topk):
    n = scores.shape[0]
    lo, hi = int(0.1 * h), int(0.9 * h)
    mask = np.zeros((h, h), np.float32)
    mask[lo:hi, lo:hi] = 1.0
    s = (scores.reshape(n, h, h) * mask).reshape(n, h * h)
    valid = s > s.mean(axis=1, keepdims=True)
    anchors = _anchors(h, stride, size)
    a_area = (anchors[:, 2] - anchors[:, 0]) * (anchors[:, 3] - anchors[:, 1])
    out = np.zeros((n, topk, 4), np.float32)
    for b in range(n):
        sb = np.where(valid[b], s[b], NEG).astype(np.float32)
        for k in range(topk):
            idx = int(np.argmax(sb))
            box = anchors[idx]
            x1 = np.maximum(box[0], anchors[:, 0]); y1 = np.maximum(box[1], anchors[:, 1])
            x2 = np.minimum(box[2], anchors[:, 2]); y2 = np.minimum(box[3], anchors[:, 3])
            inter = np.maximum(x2 - x1, 0.) * np.maximum(y2 - y1, 0.)
            a1 = (box[2] - box[0]) * (box[3] - box[1])
            iou = inter / (a1 + a_area - inter)
            sb = np.where(iou > iou_thr, NEG, sb)
            sb[idx] = NEG
            out[b, k] = box
    out[..., 0] = np.maximum(out[..., 0], 0.)
    out[..., 1] = np.maximum(out[..., 1], 0.)
    out[..., 2] = np.minimum(out[..., 2], IMG)
    out[..., 3] = np.minimum(out[..., 3], IMG)
    return out


def _roi_format(boxes):
    n, topk, _ = boxes.shape
    ids = np.broadcast_to(np.arange(n, dtype=np.float32)[:, None, None], (n, topk, 1))
    return np.concatenate([ids, boxes], axis=-1).reshape(n * topk, 5).astype(np.float32)


def _axis_matrix(n_out, start, clen):
    i = np.arange(n_out, dtype=np.float32)
    s = np.maximum((i + np.float32(0.5)) * (np.float32(clen) / np.float32(n_out)) - np.float32(0.5),
                   np.float32(0.))
    i0 = np.floor(s).astype(np.int32)
    i1 = np.minimum(i0 + 1, clen - 1)
    f = (s - i0).astype(np.float32)
    R = np.zeros((n_out, n_out), np.float32)
    np.add.at(R, (np.arange(n_out), (start + i0) % n_out), 1.0 - f)
    np.add.at(R, (np.arange(n_out), (start + i1) % n_out), f)
    return R


def host_post_means(m3, m4, m5):
    b3 = _nms_level(m3, 56, 8.0, 64.0, 0.05, 5)
    b4 = _nms_level(m4, 28, 16.0, 128.0, 0.05, 3)
    b5 = _nms_level(m5, 14, 32.0, 256.0, 0.05, 1)
    roi3, roi4, roi5 = _roi_format(b3), _roi_format(b4), _roi_format(b5)
    all_b = np.concatenate([b3, b4, b5], axis=1) / np.float32(8.0)
    x1i = np.floor(all_b[..., 0].min(1)).astype(np.int32)
    y1i = np.floor(all_b[..., 1].min(1)).astype(np.int32)
    x2i = np.floor(all_b[..., 2].max(1)).astype(np.int32)
    y2i = np.floor(all_b[..., 3].max(1)).astype(np.int32)
    RyT = np.zeros((N, HW, HW), np.float32)
    RxT2 = np.zeros((N, 112, 112), np.float32)
    for b in range(N):
        Ry = _axis_matrix(HW, int(y1i[b]), int(y2i[b] - y1i[b]))
        Rx = _axis_matrix(HW, int(x1i[b]), int(x2i[b] - x1i[b]))
        RyT[b] = Ry.T
        RxT2[b, :HW, :HW] = Rx.T
        RxT2[b, HW:, HW:] = Rx.T
    return roi3, roi4, roi5, RyT, RxT2.astype(ml_dtypes.bfloat16)


def swizzle_x2(x2):
    """[n, C, H, W] f32 -> [n, NG, 2, H, CG//2, W] (group, half, y-major)."""
    return np.ascontiguousarray(
        x2.reshape(x2.shape[0], NG, 2, CG // 2, HW, HW).transpose(0, 1, 2, 4, 3, 5))


def unswizzle_crop(cropg):
    """[n, NG, 2, H, CG//2, W] -> [n, C, H, W]."""
    n = cropg.shape[0]
    return np.ascontiguousarray(
        cropg.transpose(0, 1, 2, 4, 3, 5)).reshape(n, C, HW, HW)


_cache = {}


def _get_runners():
    if "a" not in _cache:
        _cache["a"] = SpmdRunner(build_phase_a())
        _cache["b"] = SpmdRunner(build_phase_b())
    return _cache["a"], _cache["b"]


def kernel(x2, x3, x4, x5):
    ra, rb = _get_runners()
    in_a = []
    for c in range(N_CORES):
        sl = slice(c * S_PER_CORE, (c + 1) * S_PER_CORE)
        in_a.append({
            "x3": np.ascontiguousarray(x3[sl].reshape(S_PER_CORE, C, -1)),
            "x4": np.ascontiguousarray(x4[sl].reshape(S_PER_CORE, C, -1)),
            "x5": np.ascontiguousarray(x5[sl].reshape(S_PER_CORE, C, -1)),
        })
    mo = ra.run_np(in_a)
    roi3, roi4, roi5, RyT, RxT2 = host_post_means(mo["m3"], mo["m4"], mo["m5"])
    x2g = swizzle_x2(np.asarray(x2))
    in_b = []
    for c in range(N_CORES):
        sl = slice(c * S_PER_CORE, (c + 1) * S_PER_CORE)
        in_b.append({
            "x2g": x2g[sl],
            "ryt": np.ascontiguousarray(RyT[sl]),
            "rxt2": np.ascontiguousarray(RxT2[sl]),
        })
    cropg = rb.run_np(in_b)["cropg"]
    crop = unswizzle_crop(cropg)
    return roi3, roi4, roi5, crop


# revision 5
# speedup vs baseline: 1.6967x; 1.6967x over previous
"""nn_APCNNCrop on 8 Trainium2 NeuronCores (pure data parallelism, 4 samples/core).

Pipeline:
  Phase A (device): channel means of x3/x4/x5 (exact fp32 PE reduction; DVE
        pre-adds the two 128-channel halves, then one basis matmul per sample
        routes each sample's sum to its own PSUM partition).
  Host: greedy NMS per level -> roi3/roi4/roi5, union crop window, and the
        per-sample bilinear interpolation matrices Ry/Rx (crop-resize is
        linear in x2: crop[c] = Ry @ x2[c] @ Rx^T).
  Phase B (device): crop via PE matmuls, channel-half packed at partition
        offsets 0/64 so DMAs span 120 partitions:
          MM1' (f32r): T1T[(c2,x), i] = X2ch^T @ Ry^T  (X block stationary --
                the transpose comes for free from the stationary side)
          MM2 (bf16): out[i, (c2,j)] = T1T^T @ RxT2 (block-diag Rx^T)
        Host pre-swizzles x2 to [s, group, half, y, c, x] and un-swizzles the
        crop so every DMA is contiguous per partition.

kernel(**inputs) takes FULL inputs, shards batch across cores 0-7 internally,
returns (roi3, roi4, roi5, crop) exactly like the reference forward().
"""
import numpy as np
import ml_dtypes
import jax
from jax.sharding import Mesh, PartitionSpec
from jax.experimental.shard_map import shard_map

import concourse.mybir as mybir
from concourse import bacc
from concourse.tile import TileContext
from concourse.bass2jax import _bass_exec_p, install_neuronx_cc_hook, partition_id_tensor

N_CORES = 8
S_PER_CORE = 4
N = 32
C = 256
HW = 56
CG = 64
NG = C // CG
SIZES = {"x3": 3136, "x4": 784, "x5": 196}
IMG = 447.0
NEG = np.float32(-1e30)


class SpmdRunner:
    def __init__(self, nc, n_cores=N_CORES):
        install_neuronx_cc_hook()
        self.nc = nc
        self.n_cores = n_cores
        partition_name = nc.partition_id_tensor.name if nc.partition_id_tensor else None
        in_names, out_names, out_avals, zero_outs = [], [], [], []
        for alloc in nc.m.functions[0].allocations:
            if not isinstance(alloc, mybir.MemoryLocationSet):
                continue
            name = alloc.memorylocations[0].name
            if alloc.kind == "ExternalInput":
                if name != partition_name:
                    in_names.append(name)
            elif alloc.kind == "ExternalOutput":
                out_names.append(name)
                shape = tuple(alloc.tensor_shape)
                dtype = mybir.dt.np(alloc.dtype)
                out_avals.append(jax.core.ShapedArray(shape, dtype))
                zero_outs.append(np.zeros(shape, dtype))
        self.in_names = list(in_names)
        self.out_names = list(out_names)
        n_params = len(in_names)
        all_in_names = list(in_names) + list(out_names)
        if partition_name is not None:
            all_in_names.append(partition_name)

        def _body(*args):
            operands = list(args)
            if partition_name is not None:
                operands.append(partition_id_tensor())
            outs = _bass_exec_p.bind(
                *operands,
                out_avals=tuple(out_avals),
                in_names=tuple(all_in_names),
                out_names=tuple(out_names),
                lowering_input_output_aliases=(),
                sim_require_finite=False,
                sim_require_nnan=False,
                nc=nc,
            )
            return tuple(outs)

        devices = jax.devices()[:n_cores]
        self.mesh = Mesh(np.asarray(devices), ("core",))
        in_specs = (PartitionSpec("core"),) * (n_params + len(out_names))
        out_specs = (PartitionSpec("core"),) * len(out_names)
        self.sharded = jax.jit(
            shard_map(_body, mesh=self.mesh, in_specs=in_specs,
                      out_specs=out_specs, check_rep=False),
            keep_unused=True,
        )
        self._zero_concat = [np.concatenate([z] * n_cores, axis=0) for z in zero_outs]

    def put_inputs(self, in_maps):
        sh = jax.sharding.NamedSharding(self.mesh, PartitionSpec("core"))
        args = []
        for name in self.in_names:
            concat = np.concatenate([in_maps[c][name] for c in range(self.n_cores)], axis=0)
            args.append(jax.device_put(concat, sh))
        for z in self._zero_concat:
            args.append(jax.device_put(z, sh))
        return args

    def run_np(self, in_maps):
        args = self.put_inputs(in_maps)
        outs = [np.asarray(o) for o in self.sharded(*args)]
        return {name: outs[i] for i, name in enumerate(self.out_names)}

    def time(self, in_maps, iters=8, warmup=2):
        import time as _t
        args = self.put_inputs(in_maps)
        for _ in range(warmup):
            jax.block_until_ready(self.sharded(*args))
        ts = []
        for _ in range(iters):
            t0 = _t.perf_counter()
            jax.block_until_ready(self.sharded(*args))
            ts.append(_t.perf_counter() - t0)
        return min(ts)


# ---------------------------------------------------------------- phase A
def build_phase_a(repeat=1):
    nc = bacc.Bacc("TRN2", target_bir_lowering=False, debug=False, num_devices=N_CORES)
    ins, outs = {}, {}
    for name, S in SIZES.items():
        ins[name] = nc.declare_dram_parameter(name, [S_PER_CORE, C, S], mybir.dt.float32, isOutput=False)
        outs[name] = nc.declare_dram_parameter("m" + name[1], [S_PER_CORE, S], mybir.dt.float32, isOutput=True)
    with TileContext(nc) as tc:
        with (
            tc.tile_pool(name="const", bufs=1) as cpool,
            tc.tile_pool(name="xin", bufs=2) as xpool,
            tc.tile_pool(name="acc", bufs=2) as apool,
            tc.tile_pool(name="psum", bufs=4, space="PSUM") as ppool,
        ):
            basis = []
            for s in range(S_PER_CORE):
                b = cpool.tile([128, S_PER_CORE], mybir.dt.float32, tag=f"basis{s}")
                nc.vector.memset(b[:], 0.0)
                nc.vector.memset(b[:, s:s + 1], 1.0)
                basis.append(b)

            def abody(_iv=None):
                for name, S in SIZES.items():
                    x, m = ins[name], outs[name]
                    acc = apool.tile([S_PER_CORE, S], mybir.dt.float32, tag=f"acc_{name}")
                    nc.vector.memset(acc[:], 0.0)
                    for s in range(S_PER_CORE):
                        t = xpool.tile([128, 2, S], mybir.dt.float32, tag=f"t_{name}")
                        (nc.sync if s % 2 else nc.scalar).dma_start(
                            out=t[:], in_=x[s].rearrange("(h c) s -> c h s", h=2))
                        th = xpool.tile([128, S], mybir.dt.float32, tag=f"th_{name}")
                        nc.vector.tensor_add(th[:], t[:, 0, :], t[:, 1, :])
                        for j0 in range(0, S, 512):
                            w = min(512, S - j0)
                            ps = ppool.tile([S_PER_CORE, 512], mybir.dt.float32)
                            nc.tensor.matmul(ps[:, :w], basis[s][:, :], th[:, j0:j0 + w],
                                             start=True, stop=True)
                            nc.vector.tensor_add(acc[:, j0:j0 + w], acc[:, j0:j0 + w], ps[:, :w])
                    nc.scalar.mul(out=acc[:], in_=acc[:], mul=1.0 / C)
                    nc.sync.dma_start(out=m[:, :], in_=acc[:])

            if repeat == 1:
                abody()
            else:
                with tc.For_i(0, repeat, 1) as iv:
                    abody(iv)
    nc.compile()
    return nc


# ---------------------------------------------------------------- phase B (fused_h2)
def build_phase_b(loop_n=1):
    QW = 4
    nc = bacc.Bacc("TRN2", target_bir_lowering=False, debug=False, num_devices=N_CORES)
    x2 = nc.declare_dram_parameter("x2g", [S_PER_CORE, NG, 2, HW, CG // 2, HW], mybir.dt.float32, isOutput=False)
    ryt = nc.declare_dram_parameter("ryt", [S_PER_CORE, HW, HW], mybir.dt.float32, isOutput=False)
    rxt2 = nc.declare_dram_parameter("rxt2", [S_PER_CORE, 112, 112], mybir.dt.bfloat16, isOutput=False)
    crop = nc.declare_dram_parameter("cropg", [S_PER_CORE, NG, 2, HW, CG // 2, HW], mybir.dt.float32, isOutput=True)
    with TileContext(nc) as tc:
        with (
            tc.tile_pool(name="const", bufs=1) as cpool,
            tc.tile_pool(name="xin", bufs=3) as xpool,
            tc.tile_pool(name="t1t", bufs=8) as ttpool,
            tc.tile_pool(name="osb", bufs=2) as opool,
            tc.tile_pool(name="pst", bufs=4, space="PSUM") as pstpool,
            tc.tile_pool(name="ps2", bufs=4, space="PSUM") as ps2pool,
        ):
            ryt128s, rxt2s = [], []
            for s in range(S_PER_CORE):
                r2 = cpool.tile([128, HW], mybir.dt.float32r, tag=f"ryt128_{s}")
                nc.sync.dma_start(out=r2[0:56], in_=ryt[s].bitcast(mybir.dt.float32r))
                nc.sync.dma_start(out=r2[64:120], in_=ryt[s].bitcast(mybir.dt.float32r))
                ryt128s.append(r2)
                rx = cpool.tile([112, 112], mybir.dt.bfloat16, tag=f"rxt2{s}")
                nc.sync.dma_start(out=rx[:], in_=rxt2[s])
                rxt2s.append(rx)

            def body(_iv=None):
                for s in range(S_PER_CORE):
                    for g in range(NG):
                        Xd = xpool.tile([128, CG // 2, HW], mybir.dt.float32r, tag="Xd")
                        nc.sync.dma_start(out=Xd[0:56], in_=x2[s, g, 0].bitcast(mybir.dt.float32r))
                        nc.scalar.dma_start(out=Xd[64:120], in_=x2[s, g, 1].bitcast(mybir.dt.float32r))
                        osb = opool.tile([128, CG // 2, HW], mybir.dt.float32, tag="osb")
                        for hb in range(2):
                            off = 64 * hb
                            for q in range(CG // 2 // 2 // QW):  # 4 units of QW pairs per half
                                pst = pstpool.tile([112, 56 * QW], mybir.dt.float32, tag="pstf")
                                ps2 = ps2pool.tile([128, 112 * QW], mybir.dt.float32, tag="ps2f")
                                for h in range(QW):
                                    p = QW * q + h
                                    xblk = Xd[off:off + 56, 2 * p:2 * p + 2, :].rearrange("y c x -> y (c x)")
                                    nc.tensor.matmul(pst[:, 56 * h:56 * h + 56], xblk,
                                                     ryt128s[s][off:off + 56, :],
                                                     start=True, stop=True)
                                t1t = ttpool.tile([112, 56 * QW], mybir.dt.bfloat16, tag="t1t")
                                if q % 2 == 0:
                                    nc.scalar.copy(out=t1t[:], in_=pst[:])
                                else:
                                    nc.vector.tensor_copy(t1t[:], pst[:])
                                for h in range(QW):
                                    nc.tensor.matmul(ps2[off:off + 56, 112 * h:112 * h + 112],
                                                     t1t[:, 56 * h:56 * h + 56], rxt2s[s][:],
                                                     start=True, stop=True)
                                oblk = osb[off:off + 56, 2 * QW * q:2 * QW * (q + 1), :].rearrange("i c j -> i (c j)")
                                if q % 2 == 0:
                                    nc.vector.tensor_copy(oblk, ps2[off:off + 56, :])
                                else:
                                    nc.scalar.copy(out=oblk, in_=ps2[off:off + 56, :])
                        nc.sync.dma_start(out=crop[s, g, 0], in_=osb[0:56])
                        nc.scalar.dma_start(out=crop[s, g, 1], in_=osb[64:120])

            if loop_n == 1:
                body()
            else:
                with tc.For_i(0, loop_n, 1) as iv:
                    body(iv)
    nc.compile()
    return nc


# ---------------------------------------------------------------- host NMS
def _anchors(h, stride, size):
    k = np.arange(h * h, dtype=np.int64)
    cx = (k % h).astype(np.float32) * np.float32(stride)
    cy = (k // h).astype(np.float32) * np.float32(stride)
    half = np.float32(0.5 * size)
    return np.stack([cx - half, cy - half, cx + half, cy + half], axis=1)


def _nms_level(scores, h, stride, size, iou_thr, topk):
    n = scores.shape[0]
    lo, hi = int(0.1 * h), int(0.9 * h)
    mask = np.zeros((h, h), np.float32)
    mask[lo:hi, lo:hi] = 1.0
    s = (scores.reshape(n, h, h) * mask).reshape(n, h * h)
    valid = s > s.mean(axis=1, keepdims=True)
    anchors = _anchors(h, stride, size)
    a_area = (anchors[:, 2] - anchors[:, 0]) * (anchors[:, 3] - anchors[:, 1])
    out = np.zeros((n, topk, 4), np.float32)
    for b in range(n):
        sb = np.where(valid[b], s[b], NEG).astype(np.float32)
        for k in range(topk):
            idx = int(np.argmax(sb))
            box = anchors[idx]
            x1 = np.maximum(box[0], anchors[:, 0]); y1 = np.maximum(box[1], anchors[:, 1])
            x2 = np.minimum(box[2], anchors[:, 2]); y2 = np.minimum(box[3], anchors[:, 3])
            inter = np.maximum(x2 - x1, 0.) * np.maximum(y2 - y1, 0.)
            a1 = (box[2] - box[0]) * (box[3] - box[1])
            iou = inter / (a1 + a_area - inter)
            sb = np.where(iou > iou_thr, NEG, sb)
            sb[idx] = NEG
            out[b, k] = box
    out[..., 0] = np.maximum(out[..., 0], 0.)
    out[..., 1] = np.maximum(out[..., 1], 0.)
    out[..., 2] = np.minimum(out[..., 2], IMG)
    out[..., 3] = np.minimum(out[..., 3], IMG)
    return out


def _roi_format(boxes):
    n, topk, _ = boxes.shape
    ids = np.broadcast_to(np.arange(n, dtype=np.float32)[:, None, None], (n, topk, 1))
    return np.concatenate([ids, boxes], axis=-1).reshape(n * topk, 5).astype(np.float32)


def _axis_matrix(n_out, start, clen):
    i = np.arange(n_out, dtype=np.float32)
    s = np.maximum((i + np.float32(0.5)) * (np.float32(clen) / np.float32(n_out)) - np.float32(0.5),
                   np.float32(0.))
    i0 = np.floor(s).astype(np.int32)
    i1 = np.minimum(i0 + 1, clen - 1)
    f = (s - i0).astype(np.float32)
    R = np.zeros((n_out, n_out), np.float32)
    np.add.at(R, (np.arange(n_out), (start + i0) % n_out), 1.0 - f)
    np.add.at(R, (np.arange(n_out), (start + i1) % n_out), f)
    return R


def host_post_means(m3, m4, m5):
    b3 = _nms_level(m3, 56, 8.0, 64.0, 0.05, 5)
    b4 = _nms_level(m4, 28, 16.0, 128.0, 0.05, 3)
    b5 = _nms_level(m5, 14, 32.0, 256.0, 0.05, 1)
    roi3, roi4, roi5 = _roi_format(b3), _roi_format(b4), _roi_format(b5)
    all_b = np.concatenate([b3, b4, b5], axis=1) / np.float32(8.0)
    x1i = np.floor(all_b[..., 0].min(1)).astype(np.int32)
    y1i = np.floor(all_b[..., 1].min(1)).astype(np.int32)
    x2i = np.floor(all_b[..., 2].max(1)).astype(np.int32)
    y2i = np.floor(all_b[..., 3].max(1)).astype(np.int32)
    RyT = np.zeros((N, HW, HW), np.float32)
    RxT2 = np.zeros((N, 112, 112), np.float32)
    for b in range(N):
        Ry = _axis_matrix(HW, int(y1i[b]), int(y2i[b] - y1i[b]))
        Rx = _axis_matrix(HW, int(x1i[b]), int(x2i[b] - x1i[b]))
        RyT[b] = Ry.T
        RxT2[b, :HW, :HW] = Rx.T
        RxT2[b, HW:, HW:] = Rx.T
    return roi3, roi4, roi5, RyT, RxT2.astype(ml_dtypes.bfloat16)


def swizzle_x2(x2):
    """[n, C, H, W] f32 -> [n, NG, 2, H, CG//2, W] (group, half, y-major)."""
    return np.ascontiguousarray(
        x2.reshape(x2.shape[0], NG, 2, CG // 2, HW, HW).transpose(0, 1, 2, 4, 3, 5))


def unswizzle_crop(cropg):
    """[n, NG, 2, H, CG//2, W] -> [n, C, H, W]."""
    n = cropg.shape[0]
    return np.ascontiguousarray(
        cropg.transpose(0, 1, 2, 4, 3, 5)).reshape(n, C, HW, HW)


_cache = {}


def _get_runners():
    if "a" not in _cache:
        _cache["a"] = SpmdRunner(build_phase_a())
        _cache["b"] = SpmdRunner(build_phase_b())
    return _cache["a"], _cache["b"]


def kernel(x2, x3, x4, x5):
    ra, rb = _get_runners()
    in_a = []
    for c in range(N_CORES):
        sl = slice(c * S_PER_CORE, (c + 1) * S_PER_CORE)
        in_a.append({
            "x3": np.ascontiguousarray(x3[sl].reshape(S_PER_CORE, C, -1)),
            "x4": np.ascontiguousarray(x4[sl].reshape(S_PER_CORE, C, -1)),
            "x5": np.ascontiguousarray(x5[sl].reshape(S_PER_CORE, C, -1)),
        })
    mo = ra.run_np(in_a)
    roi3, roi4, roi5, RyT, RxT2 = host_post_means(mo["m3"], mo["m4"], mo["m5"])
    x2g = swizzle_x2(np.asarray(x2))
    in_b = []
    for c in range(N_CORES):
        sl = slice(c * S_PER_CORE, (c + 1) * S_PER_CORE)
        in_b.append({
            "x2g": x2g[sl],
            "ryt": np.ascontiguousarray(RyT[sl]),
            "rxt2": np.ascontiguousarray(RxT2[sl]),
        })
    cropg = rb.run_np(in_b)["cropg"]
    crop = unswizzle_crop(cropg)
    return roi3, roi4, roi5, crop
